# revision 13
# baseline (speedup 1.0000x reference)
"""Trainium2 Bass kernel for nn_ExternalInteraction.

Math (per batch b):
    img_sum[d]  = sum_i image[b, i, d]
    user_sum[d] = sum_u user[b, u, d]
    out_user[b, u, d] = user[b, u, d] * img_sum[d]
    out_img[b, i, d]  = image[b, i, d] * user_sum[d]

Shapes: user [32, 128, 256] f32, image [32, 256, 256] f32.
Sharding: data-parallel over batch, 4 batches per core across 8 cores.

Per-core kernel layout: U/I on the partition dim, D on the free dim. The
img batch [256, 256] is viewed flat as [128, 2, 256] (i = 2p + j), so each
partition's DMA run is 2 KB contiguous. The partition-dim reduction AND
the broadcast back across partitions are fused into a single TensorE
matmul with an all-ones [128, 128] stationary operand:
out[p, d] = sum_k ones[k, p] * x[k, d] = sum_k x[k, d] for every partition.
VectorE pre-reduces the img j-halves and does the elementwise multiplies.
"""

import numpy as np

B, U, I, D = 32, 128, 256, 256
NCORES = 8
BPC = B // NCORES  # batches per core
P = 128

_compiled = None


def _skip_const_ap_memsets():
    """The Bass() constructor memsets four unused const-AP tiles on GpSimd;
    they gate the entry barrier and push the first DMA out. None of the ops
    this kernel uses read const_aps, so drop those memsets."""
    import concourse.bass as bassmod

    if getattr(bassmod, "_const_memset_patched", False):
        return
    orig = bassmod.BassGpSimd.memset

    def memset(self, ap, constant):
        t = getattr(ap, "tensor", None)
        if t is not None and str(getattr(t, "name", "")).startswith("const-"):
            return None
        return orig(self, ap, constant)

    bassmod.BassGpSimd.memset = memset
    bassmod._const_memset_patched = True


def _build():
    import concourse.bacc as bacc
    import concourse.mybir as mybir
    import concourse.tile as tile

    f32 = mybir.dt.float32
    nc = bacc.Bacc("TRN2", target_bir_lowering=False, debug=False, num_devices=NCORES)

    user = nc.dram_tensor("user", [BPC, U, D], f32, kind="ExternalInput")
    img = nc.dram_tensor("img", [BPC, P, 2, D], f32, kind="ExternalInput")
    out_user = nc.dram_tensor("out_user", [BPC, U, D], f32, kind="ExternalOutput")
    out_img = nc.dram_tensor("out_img", [BPC, P, 2, D], f32, kind="ExternalOutput")

    with tile.TileContext(nc) as tc:
        with (
            tc.tile_pool(name="const", bufs=1) as cpool,
            tc.tile_pool(name="io", bufs=BPC) as io,
            tc.tile_pool(name="psum", bufs=BPC, space="PSUM") as psum,
        ):
            ones = cpool.tile([P, P], f32)
            nc.vector.memset(ones[:], 1.0)

            u_t, i_t = {}, {}
            for b in range(BPC):
                # alternate the two HWDGE rings for loads
                ld = nc.sync if b % 2 == 0 else nc.scalar
                u_t[b] = io.tile([P, D], f32, tag="u", name=f"u{b}")
                i_t[b] = io.tile([P, 2, D], f32, tag="i", name=f"i{b}")
                if b == 0:
                    # user first: primes the PE (usum0 needs no DVE pre-add)
                    ld.dma_start(u_t[b][:], user[b])
                    ld.dma_start(i_t[b][:], img[b])
                else:
                    ld.dma_start(i_t[b][:], img[b])
                    ld.dma_start(u_t[b][:], user[b])

            for b in range(BPC):
                st = nc.scalar if b % 2 == 0 else nc.sync
                # pre-reduce the two j-halves of img, then one matmul for
                # the partition-sum + broadcast; batch 0 runs usum first to
                # prime the PE before the first pre-add lands
                ired = io.tile([P, D], f32, tag="ired", name=f"ired{b}")
                usum = psum.tile([P, D], f32, tag="ub", name=f"ub{b}")
                isum = psum.tile([P, D], f32, tag="ib", name=f"ib{b}")
                if b == 0:
                    nc.tensor.matmul(usum[:], ones[:], u_t[b][:], start=True, stop=True)
                nc.vector.tensor_tensor(
                    ired[:], i_t[b][:, 0, :], i_t[b][:, 1, :], mybir.AluOpType.add
                )
                nc.tensor.matmul(isum[:], ones[:], ired[:], start=True, stop=True)
                if b != 0:
                    nc.tensor.matmul(usum[:], ones[:], u_t[b][:], start=True, stop=True)

                ou_t = io.tile([P, D], f32, tag="ou", name=f"ou{b}")
                oi_t = io.tile([P, 2, D], f32, tag="oi", name=f"oi{b}")
                nc.vector.tensor_tensor(
                    oi_t[:],
                    i_t[b][:],
                    usum[:, None, :].to_broadcast([P, 2, D]),
                    mybir.AluOpType.mult,
                )
                nc.vector.tensor_tensor(
                    ou_t[:], u_t[b][:], isum[:], mybir.AluOpType.mult
                )
                st.dma_start(out_img[b], oi_t[:])
                st.dma_start(out_user[b], ou_t[:])

    nc.compile()
    return nc


def kernel(user_attributes, image_attributes, _trace=False):
    global _compiled
    from concourse import bass_utils

    if _compiled is None:
        _compiled = _build()
    nc = _compiled

    ua = np.ascontiguousarray(np.asarray(user_attributes, dtype=np.float32))
    ia = np.ascontiguousarray(np.asarray(image_attributes, dtype=np.float32))
    ua_s = ua.reshape(NCORES, BPC, U, D)
    ia_s = ia.reshape(NCORES, BPC, P, 2, D)

    in_maps = [{"user": ua_s[c], "img": ia_s[c]} for c in range(NCORES)]
    res = bass_utils.run_bass_kernel_spmd(
        nc, in_maps, core_ids=list(range(NCORES)), trace=_trace
    )
    out_user = np.concatenate([res.results[c]["out_user"] for c in range(NCORES)], axis=0)
    out_img = np.concatenate(
        [res.results[c]["out_img"].reshape(BPC, I, D) for c in range(NCORES)], axis=0
    )
    if _trace:
        kernel._last_results = res
    return (out_user, out_img)


# revision 14
# speedup vs baseline: 1.0409x; 1.0409x over previous
"""Trainium2 Bass kernel for nn_ExternalInteraction.

Math (per batch b):
    img_sum[d]  = sum_i image[b, i, d]
    user_sum[d] = sum_u user[b, u, d]
    out_user[b, u, d] = user[b, u, d] * img_sum[d]
    out_img[b, i, d]  = image[b, i, d] * user_sum[d]

Shapes: user [32, 128, 256] f32, image [32, 256, 256] f32.
Sharding: data-parallel over batch, 4 batches per core across 8 cores.

Per-core kernel layout: U/I on the partition dim, D on the free dim. The
img batch [256, 256] is viewed flat as [128, 2, 256] (i = 2p + j), so each
partition's DMA run is 2 KB contiguous. The partition-dim reduction AND
the broadcast back across partitions are fused into a single TensorE
matmul with an all-ones [128, 128] stationary operand:
out[p, d] = sum_k ones[k, p] * x[k, d] = sum_k x[k, d] for every partition.
VectorE pre-reduces the img j-halves and does the elementwise multiplies.
"""

import numpy as np

B, U, I, D = 32, 128, 256, 256
NCORES = 8
BPC = B // NCORES  # batches per core
P = 128

_compiled = None


def _skip_const_ap_memsets():
    """The Bass() constructor memsets four unused const-AP tiles on GpSimd;
    they gate the entry barrier and push the first DMA out. None of the ops
    this kernel uses read const_aps, so drop those memsets."""
    import concourse.bass as bassmod

    if getattr(bassmod, "_const_memset_patched", False):
        return
    orig = bassmod.BassGpSimd.memset

    def memset(self, ap, constant):
        t = getattr(ap, "tensor", None)
        if t is not None and str(getattr(t, "name", "")).startswith("const-"):
            return None
        return orig(self, ap, constant)

    bassmod.BassGpSimd.memset = memset
    bassmod._const_memset_patched = True


def _build():
    import concourse.bacc as bacc
    import concourse.mybir as mybir
    import concourse.tile as tile

    f32 = mybir.dt.float32
    nc = bacc.Bacc("TRN2", target_bir_lowering=False, debug=False, num_devices=NCORES)

    user = nc.dram_tensor("user", [BPC, U, D], f32, kind="ExternalInput")
    img = nc.dram_tensor("img", [BPC, P, 2, D], f32, kind="ExternalInput")
    out_user = nc.dram_tensor("out_user", [BPC, U, D], f32, kind="ExternalOutput")
    out_img = nc.dram_tensor("out_img", [BPC, P, 2, D], f32, kind="ExternalOutput")

    with tile.TileContext(nc) as tc:
        with (
            tc.tile_pool(name="const", bufs=1) as cpool,
            tc.tile_pool(name="io", bufs=BPC) as io,
            tc.tile_pool(name="psum", bufs=BPC, space="PSUM") as psum,
        ):
            ones = cpool.tile([P, P], f32)
            nc.vector.memset(ones[:], 1.0)

            u_t, i_t = {}, {}
            for b in range(BPC):
                # alternate the two HWDGE rings for loads
                ld = nc.sync if b % 2 == 0 else nc.scalar
                u_t[b] = io.tile([P, D], f32, tag="u", name=f"u{b}")
                i_t[b] = io.tile([P, 2, D], f32, tag="i", name=f"i{b}")
                if b == 0:
                    # user first: primes the PE (usum0 needs no DVE pre-add)
                    ld.dma_start(u_t[b][:], user[b])
                    ld.dma_start(i_t[b][:], img[b])
                else:
                    ld.dma_start(i_t[b][:], img[b])
                    ld.dma_start(u_t[b][:], user[b])

            for b in range(BPC):
                st = nc.scalar if b % 2 == 0 else nc.sync
                # pre-reduce the two j-halves of img, then one matmul for
                # the partition-sum + broadcast; batch 0 runs usum first to
                # prime the PE before the first pre-add lands
                ired = io.tile([P, D], f32, tag="ired", name=f"ired{b}")
                usum = psum.tile([P, D], f32, tag="ub", name=f"ub{b}")
                isum = psum.tile([P, D], f32, tag="ib", name=f"ib{b}")
                if b == 0:
                    nc.tensor.matmul(usum[:], ones[:], u_t[b][:], start=True, stop=True)
                nc.vector.tensor_tensor(
                    ired[:], i_t[b][:, 0, :], i_t[b][:, 1, :], mybir.AluOpType.add
                )
                nc.tensor.matmul(isum[:], ones[:], ired[:], start=True, stop=True)
                if b != 0:
                    nc.tensor.matmul(usum[:], ones[:], u_t[b][:], start=True, stop=True)

                ou_t = io.tile([P, D], f32, tag="ou", name=f"ou{b}")
                oi_t = io.tile([P, 2, D], f32, tag="oi", name=f"oi{b}")
                nc.vector.tensor_tensor(
                    oi_t[:],
                    i_t[b][:],
                    usum[:, None, :].to_broadcast([P, 2, D]),
                    mybir.AluOpType.mult,
                )
                nc.vector.tensor_tensor(
                    ou_t[:], u_t[b][:], isum[:], mybir.AluOpType.mult
                )
                st.dma_start(out_img[b], oi_t[:])
                st.dma_start(out_user[b], ou_t[:])

    nc.compile()
    return nc


def kernel(user_attributes, image_attributes, _trace=False):
    global _compiled
    from concourse import bass_utils

    if _compiled is None:
        _compiled = _build()
    nc = _compiled

    ua = np.ascontiguousarray(np.asarray(user_attributes, dtype=np.float32))
    ia = np.ascontiguousarray(np.asarray(image_attributes, dtype=np.float32))
    ua_s = ua.reshape(NCORES, BPC, U, D)
    ia_s = ia.reshape(NCORES, BPC, P, 2, D)

    in_maps = [{"user": ua_s[c], "img": ia_s[c]} for c in range(NCORES)]
    kw = {"trace_cores": list(range(NCORES))} if _trace else {}
    res = bass_utils.run_bass_kernel_spmd(
        nc, in_maps, core_ids=list(range(NCORES)), trace=_trace, **kw
    )
    out_user = np.concatenate([res.results[c]["out_user"] for c in range(NCORES)], axis=0)
    out_img = np.concatenate(
        [res.results[c]["out_img"].reshape(BPC, I, D) for c in range(NCORES)], axis=0
    )
    if _trace:
        kernel._last_results = res
    return (out_user, out_img)


# revision 23
# speedup vs baseline: 1.1396x; 1.0947x over previous
"""Trainium2 Bass kernel for nn_ExternalInteraction.

Math (per batch b):
    img_sum[d]  = sum_i image[b, i, d]
    user_sum[d] = sum_u user[b, u, d]
    out_user[b, u, d] = user[b, u, d] * img_sum[d]
    out_img[b, i, d]  = image[b, i, d] * user_sum[d]

Shapes: user [32, 128, 256] f32, image [32, 256, 256] f32.
Sharding: data-parallel over batch, 4 batches per core across 8 cores.

Per-core kernel layout: U/I on the partition dim, D on the free dim. The
img batch [256, 256] is viewed flat as [128, 2, 256] (i = 2p + j), so each
partition's DMA run is 2 KB contiguous. The partition-dim reduction AND
the broadcast back across partitions are fused into a single TensorE
matmul with an all-ones [128, 128] stationary operand:
out[p, d] = sum_k ones[k, p] * x[k, d] = sum_k x[k, d] for every partition.
VectorE pre-reduces the img j-halves and does the elementwise multiplies.
"""

import numpy as np

B, U, I, D = 32, 128, 256, 256
NCORES = 8
BPC = B // NCORES  # batches per core
P = 128

_compiled = None


def _skip_const_ap_memsets():
    """The Bass() constructor memsets four unused const-AP tiles on GpSimd;
    they gate the entry barrier and push the first DMA out. None of the ops
    this kernel uses read const_aps, so drop those memsets."""
    import concourse.bass as bassmod

    if getattr(bassmod, "_const_memset_patched", False):
        return
    orig = bassmod.BassGpSimd.memset

    def memset(self, ap, constant):
        t = getattr(ap, "tensor", None)
        if t is not None and str(getattr(t, "name", "")).startswith("const-"):
            return None
        return orig(self, ap, constant)

    bassmod.BassGpSimd.memset = memset
    bassmod._const_memset_patched = True


def _build():
    import concourse.bacc as bacc
    import concourse.mybir as mybir
    import concourse.tile as tile

    f32 = mybir.dt.float32
    nc = bacc.Bacc("TRN2", target_bir_lowering=False, debug=False, num_devices=NCORES)

    user = nc.dram_tensor("user", [BPC, U, D], f32, kind="ExternalInput")
    img = nc.dram_tensor("img", [BPC, P, 2, D], f32, kind="ExternalInput")
    out_user = nc.dram_tensor("out_user", [BPC, U, D], f32, kind="ExternalOutput")
    out_img = nc.dram_tensor("out_img", [BPC, P, 2, D], f32, kind="ExternalOutput")

    with tile.TileContext(nc) as tc:
        with (
            tc.tile_pool(name="const", bufs=1) as cpool,
            tc.tile_pool(name="io", bufs=BPC) as io,
            tc.tile_pool(name="psum", bufs=BPC, space="PSUM") as psum,
        ):
            ones_t = cpool.tile([P, P], f32)
            nc.vector.memset(ones_t[:], 1.0)
            ones = ones_t[:]

            u_t, i_t = {}, {}
            for b in range(BPC):
                # alternate the two HWDGE rings for loads
                ld = nc.sync if b % 2 == 0 else nc.scalar
                u_t[b] = io.tile([P, D], f32, tag="u", name=f"u{b}")
                i_t[b] = io.tile([P, 2, D], f32, tag="i", name=f"i{b}")
                ld.dma_start(i_t[b][:], img[b])
                ld.dma_start(u_t[b][:], user[b])

            for b in range(BPC):
                st = nc.scalar if b % 2 == 0 else nc.sync
                # pre-reduce the two j-halves of img, then one matmul for
                # the partition-sum + broadcast; batch 0 runs usum first to
                # prime the PE before the first pre-add lands
                ired = io.tile([P, D], f32, tag="ired", name=f"ired{b}")
                usum = psum.tile([P, D], f32, tag="ub", name=f"ub{b}")
                isum = psum.tile([P, D], f32, tag="ib", name=f"ib{b}")
                nc.vector.tensor_tensor(
                    ired[:], i_t[b][:, 0, :], i_t[b][:, 1, :], mybir.AluOpType.add
                )
                nc.tensor.matmul(isum[:], ones, ired[:], start=True, stop=True)
                nc.tensor.matmul(usum[:], ones, u_t[b][:], start=True, stop=True)

                ou_t = io.tile([P, D], f32, tag="ou", name=f"ou{b}")
                oi_t = io.tile([P, 2, D], f32, tag="oi", name=f"oi{b}")
                nc.vector.tensor_tensor(
                    ou_t[:], u_t[b][:], isum[:], mybir.AluOpType.mult
                )
                nc.vector.tensor_tensor(
                    oi_t[:],
                    i_t[b][:],
                    usum[:, None, :].to_broadcast([P, 2, D]),
                    mybir.AluOpType.mult,
                )
                st.dma_start(out_img[b], oi_t[:])
                st.dma_start(out_user[b], ou_t[:])

    nc.compile()
    return nc


def kernel(user_attributes, image_attributes, _trace=False):
    global _compiled
    from concourse import bass_utils

    if _compiled is None:
        _compiled = _build()
    nc = _compiled

    ua = np.ascontiguousarray(np.asarray(user_attributes, dtype=np.float32))
    ia = np.ascontiguousarray(np.asarray(image_attributes, dtype=np.float32))
    ua_s = ua.reshape(NCORES, BPC, U, D)
    ia_s = ia.reshape(NCORES, BPC, P, 2, D)

    in_maps = [{"user": ua_s[c], "img": ia_s[c]} for c in range(NCORES)]
    kw = {"trace_cores": list(range(NCORES))} if _trace else {}
    res = bass_utils.run_bass_kernel_spmd(
        nc, in_maps, core_ids=list(range(NCORES)), trace=_trace, **kw
    )
    out_user = np.concatenate([res.results[c]["out_user"] for c in range(NCORES)], axis=0)
    out_img = np.concatenate(
        [res.results[c]["out_img"].reshape(BPC, I, D) for c in range(NCORES)], axis=0
    )
    if _trace:
        kernel._last_results = res
    return (out_user, out_img)
